# revision 4
# baseline (speedup 1.0000x reference)
"""Bit2Num dequantization kernel for Trainium2 (Bass/Tile), SPMD over 8 cores.

Reference computation (B=4):
    bits = x.reshape(batch, 2048, 4)                # x in {0,1} stored fp32
    num  = sum_b bits[..., b] * 2**(3-b)            # weights [8,4,2,1]
    out  = (num + 0.5) / 16
        = 0.5*x0 + 0.25*x1 + 0.125*x2 + 0.0625*x3 + 0.03125

Sharding: batch (16384) split evenly across 8 NeuronCores; pure data
parallel, no collectives.

Per-core kernel: 16 stripes of [128 rows x 8192 cols]. Each stripe is one
contiguous 4MB DMA load; the 4 bit-streams are strided SBUF views
(stride 4). Compute is a Horner chain:
    s3 = 0.0625 * x3                      (ScalarE, free affine)
    u  = (x2 * 0.125 + 0.03125) + s3      (VectorE AFFINE_THEN_ADD)
    v  = (x1 * 0.25) + u                  (VectorE AFFINE_THEN_ADD)
    o  = (x0 * 0.5)  + v                  (VectorE AFFINE_THEN_ADD)
All values are dyadic rationals representable exactly in fp32, so the
result is bit-exact vs the reference.
"""

import numpy as np

BATCH = 16384
N_SYM = 2048
NBITS = 4
COLS = N_SYM * NBITS  # 8192
N_CORES = 8
ROWS_PER_CORE = BATCH // N_CORES  # 2048
P = 128  # SBUF partitions

_NC_CACHE = {}


DEFAULT_CHUNK = 8192


DEFAULT_STRUCTURE = "b16a2"
DEFAULT_OUT_DMA = "alt"


def _build_program(
    col_chunk=DEFAULT_CHUNK,
    repeats=1,
    structure=DEFAULT_STRUCTURE,
    in_bufs=None,
    mid_bufs=3,
    out_bufs=3,
    out_dma=DEFAULT_OUT_DMA,
):
    """Build the per-core Bass program (identical on every core).

    repeats>1 re-runs the whole computation N times inside one NEFF —
    used only for benchmarking (launch overhead cancels in T(N)-T(1))."""
    import concourse.mybir as mybir
    from concourse import bacc
    from concourse.tile import TileContext

    # Bacc (not raw Bass): its compile() pass splits multi-sem waits into
    # event-semaphore chains (TRN2 allows max 1 wait/instruction) and runs
    # codegen for extended-ISA instructions (the custom DVE op below).
    nc = bacc.Bacc("TRN2")
    f32 = mybir.dt.float32
    # "+o16" suffix: store the output as bf16. Every output value is
    # (2k+1)/32 for k in 0..15 — 5 significand bits, exact in bf16 — so
    # this halves output HBM traffic with zero numeric error. The host
    # upconverts to fp32 after the gather.
    structure, _, _osuf = structure.partition("+")
    out_dt = mybir.dt.bfloat16 if _osuf == "o16" else f32
    x = nc.dram_tensor("x", [ROWS_PER_CORE, COLS], f32, kind="ExternalInput")
    out = nc.dram_tensor("out", [ROWS_PER_CORE, N_SYM], out_dt, kind="ExternalOutput")

    n_stripes = ROWS_PER_CORE // P  # 16
    chunks_per_stripe = COLS // col_chunk
    sym_chunk = col_chunk // NBITS
    Copy = mybir.ActivationFunctionType.Copy
    if in_bufs is None:
        in_bufs = 3

    def out_eng(idx):
        if out_dma == "alt":
            return nc.scalar if idx % 2 == 0 else nc.sync
        return {"sync": nc.sync, "scalar": nc.scalar}[out_dma]

    if structure == "noop":
        # minimal program: one tiny round trip, for launch-overhead probes
        with TileContext(nc) as tc:
            with tc.tile_pool(name="p", bufs=1) as pool:
                t = pool.tile([P, 128], f32)
                nc.sync.dma_start(out=t, in_=x[0:P, 0:128])
                nc.sync.dma_start(out=out[0:P, 0:128], in_=t)
        nc.finalize()
        return nc

    with TileContext(nc) as tc:
        with (
            tc.tile_pool(name="inp", bufs=in_bufs) as in_pool,
            tc.tile_pool(name="mid", bufs=mid_bufs) as mid_pool,
            tc.tile_pool(name="outp", bufs=out_bufs) as out_pool,
        ):
            for it, i in enumerate(
                [s for _ in range(repeats) for s in range(n_stripes)]
            ):
                for c in range(chunks_per_stripe):
                    xt = in_pool.tile([P, col_chunk], f32, tag="xt")
                    nc.sync.dma_start(
                        out=xt,
                        in_=x[i * P : (i + 1) * P, c * col_chunk : (c + 1) * col_chunk],
                    )
                    xb = xt.rearrange("p (s b) -> p s b", b=NBITS)
                    x0, x1, x2, x3 = (xb[:, :, b] for b in range(NBITS))
                    o = out_pool.tile([P, sym_chunk], out_dt, tag="o")

                    if structure == "chain3":
                        # Horner: w = x0 + x1/2 + x2/4 + x3/8 (3x custom DVE),
                        # then o = w/2 + 1/32 on ScalarE.
                        u = mid_pool.tile([P, sym_chunk], f32, tag="u")
                        nc.vector.affine_then_add(
                            out=u, in0=x3, in1=x2, scale=0.5, bias=0.0
                        )
                        v = mid_pool.tile([P, sym_chunk], f32, tag="v")
                        nc.vector.affine_then_add(
                            out=v, in0=u, in1=x1, scale=0.5, bias=0.0
                        )
                        w = mid_pool.tile([P, sym_chunk], f32, tag="w")
                        nc.vector.affine_then_add(
                            out=w, in0=v, in1=x0, scale=0.5, bias=0.0
                        )
                        nc.scalar.activation(o, w, Copy, bias=0.03125, scale=0.5)
                    elif structure == "act1":
                        # ACT prescales x3 (incl. the +1/32), DVE chain ends
                        # at o directly — no final dense pass.
                        s3 = mid_pool.tile([P, sym_chunk], f32, tag="s3")
                        nc.scalar.activation(s3, x3, Copy, bias=0.03125, scale=0.0625)
                        u = mid_pool.tile([P, sym_chunk], f32, tag="u")
                        nc.vector.affine_then_add(
                            out=u, in0=x2, in1=s3, scale=0.125, bias=0.0
                        )
                        v = mid_pool.tile([P, sym_chunk], f32, tag="v")
                        nc.vector.affine_then_add(
                            out=v, in0=x1, in1=u, scale=0.25, bias=0.0
                        )
                        nc.vector.affine_then_add(
                            out=o, in0=x0, in1=v, scale=0.5, bias=0.0
                        )
                    elif structure == "act1ip":
                        # act1 but the DVE chain accumulates in place in one
                        # tile (one mid tag; less SBUF, fewer tile releases)
                        acc = mid_pool.tile([P, sym_chunk], f32, tag="acc")
                        nc.scalar.activation(acc, x3, Copy, bias=0.03125, scale=0.0625)
                        nc.vector.affine_then_add(
                            out=acc, in0=x2, in1=acc, scale=0.125, bias=0.0
                        )
                        nc.vector.affine_then_add(
                            out=acc, in0=x1, in1=acc, scale=0.25, bias=0.0
                        )
                        nc.vector.affine_then_add(
                            out=o, in0=x0, in1=acc, scale=0.5, bias=0.0
                        )
                    elif structure == "b16a3":
                        # Exact-bf16 intermediates: ACT prescales 3 streams
                        # (strided fp32 -> dense bf16), DVE combines with two
                        # 2x-mode bf16 adds + one fp32 affine. All values are
                        # dyadic rationals representable exactly in bf16.
                        bf16 = mybir.dt.bfloat16
                        s3 = mid_pool.tile([P, sym_chunk], bf16, tag="s3")
                        nc.scalar.activation(s3, x3, Copy, bias=0.03125, scale=0.0625)
                        s2 = mid_pool.tile([P, sym_chunk], bf16, tag="s2")
                        nc.scalar.activation(s2, x2, Copy, bias=0.0, scale=0.125)
                        s1 = mid_pool.tile([P, sym_chunk], bf16, tag="s1")
                        nc.scalar.activation(s1, x1, Copy, bias=0.0, scale=0.25)
                        u = mid_pool.tile([P, sym_chunk], bf16, tag="u")
                        nc.vector.tensor_add(out=u, in0=s2, in1=s3)
                        v = mid_pool.tile([P, sym_chunk], bf16, tag="v")
                        nc.vector.tensor_add(out=v, in0=u, in1=s1)
                        nc.vector.affine_then_add(
                            out=o, in0=x0, in1=v, scale=0.5, bias=0.0
                        )
                    elif structure == "b16a2":
                        # 2 ACT prescales, DVE: bf16 add + 2 affines
                        bf16 = mybir.dt.bfloat16
                        s3 = mid_pool.tile([P, sym_chunk], bf16, tag="s3")
                        nc.scalar.activation(s3, x3, Copy, bias=0.03125, scale=0.0625)
                        s2 = mid_pool.tile([P, sym_chunk], bf16, tag="s2")
                        nc.scalar.activation(s2, x2, Copy, bias=0.0, scale=0.125)
                        u = mid_pool.tile([P, sym_chunk], bf16, tag="u")
                        nc.vector.tensor_add(out=u, in0=s2, in1=s3)
                        v = mid_pool.tile([P, sym_chunk], bf16, tag="v")
                        nc.vector.affine_then_add(
                            out=v, in0=x1, in1=u, scale=0.25, bias=0.0
                        )
                        nc.vector.affine_then_add(
                            out=o, in0=x0, in1=v, scale=0.5, bias=0.0
                        )
                    elif structure == "poolsplit":
                        # 2 ACT prescales + 1 GPSIMD add + 2 DVE affines.
                        s3 = mid_pool.tile([P, sym_chunk], f32, tag="s3")
                        nc.scalar.activation(s3, x3, Copy, bias=0.03125, scale=0.0625)
                        s2 = mid_pool.tile([P, sym_chunk], f32, tag="s2")
                        nc.scalar.activation(s2, x2, Copy, bias=0.0, scale=0.125)
                        p = mid_pool.tile([P, sym_chunk], f32, tag="p")
                        nc.gpsimd.tensor_tensor(p, s2, s3, mybir.AluOpType.add)
                        v = mid_pool.tile([P, sym_chunk], f32, tag="v")
                        nc.vector.affine_then_add(
                            out=v, in0=x1, in1=p, scale=0.25, bias=0.0
                        )
                        nc.vector.affine_then_add(
                            out=o, in0=x0, in1=v, scale=0.5, bias=0.0
                        )
                    elif structure == "dma_only":
                        # bandwidth floor probe: no compute, garbage output
                        o = xt[:, 0:sym_chunk]
                    else:
                        raise ValueError(structure)

                    out_eng(it * chunks_per_stripe + c).dma_start(
                        out=out[
                            i * P : (i + 1) * P, c * sym_chunk : (c + 1) * sym_chunk
                        ],
                        in_=o,
                    )

    nc.finalize()
    return nc


def _get_nc(col_chunk=DEFAULT_CHUNK, structure=DEFAULT_STRUCTURE):
    key = (col_chunk, structure)
    if key not in _NC_CACHE:
        _NC_CACHE[key] = _build_program(col_chunk, structure=structure)
    return _NC_CACHE[key]


def run(x, trace=False, col_chunk=DEFAULT_CHUNK, structure=DEFAULT_STRUCTURE):
    """Run the SPMD kernel; returns (full_output, BassKernelResults)."""
    from concourse.bass_utils import run_bass_kernel_spmd

    x = np.asarray(x, dtype=np.float32)
    assert x.shape == (BATCH, COLS), x.shape
    nc = _get_nc(col_chunk, structure)
    shards = np.split(x, N_CORES, axis=0)
    in_maps = [{"x": np.ascontiguousarray(s)} for s in shards]
    res = run_bass_kernel_spmd(
        nc, in_maps, core_ids=list(range(N_CORES)), trace=trace
    )
    out = np.concatenate([r["out"] for r in res.results], axis=0)
    if out.dtype != np.float32:
        out = out.astype(np.float32)  # bf16 -> fp32 is exact for these values
    return out, res


def kernel(x, B=4, **_ignored):
    assert int(B) == NBITS
    out, _ = run(x, trace=False)
    return out

